# revision 6
# baseline (speedup 1.0000x reference)
"""MoE gate (sigmoid scores + grouped top-k routing) on 8 Trainium2 cores.

Reference computation (per token):
    scores = sigmoid(x @ W.T)                  # [T, 256]
    s = scores + bias                          # selection scores
    group_score[g] = sum(top2(s[g*32:(g+1)*32]))
    keep top-4 groups, mask the rest to -inf
    idx = top8(masked s)                       # [T, 8] int32, descending
    w = scores[idx]; w = w / w.sum() * 2.5     # [T, 8] f32

Sharding: tokens split 8 ways (2048/core); W/bias replicated. Host
pre-shuffles x and W into the transposed tiled layout the TensorE needs
(contraction dim on partitions) so the device does no transposes.

Device per 128-token tile: 56 accumulating matmuls (K=128 each) into
PSUM -> sigmoid on ScalarE -> routing (grouped top-2 via segmented
reduce_max + match_replace, group top-4 via max8, expert top-8 via
max8/find_index8) on VectorE. Final score gather is 8 fused
compare-multiply-accumulate ops.
"""

import os

import numpy as np

import concourse.bass as bass
import concourse.mybir as mybir
import concourse.tile as tile
from concourse import bacc
from concourse.bass_utils import run_bass_kernel_spmd

T = 16384
DIM = 7168
E = 256
G = 8
EPG = E // G          # 32 experts per group
TOPKG = 4
TOPK = 8
SCALE = 2.5
NCORES = 8
TPC = T // NCORES     # 2048 tokens per core
P = 128
NT = TPC // P         # 16 token tiles per core
KT = DIM // P         # 56 contraction tiles
NEG = -1.0e30

# matmul precision: 'fp32r' (1 cyc/row, tf32-ish), 'bf16x3' (3 cyc/row,
# near-fp32), 'fp32' (4 cyc/row, exact)
MODE = os.environ.get("GATE_KERNEL_MODE", "bf16x3")

f32 = mybir.dt.float32
f32r = mybir.dt.float32r
bf16 = mybir.dt.bfloat16
f16 = mybir.dt.float16
f8e4 = mybir.dt.float8e4
i32 = mybir.dt.int32
u16 = mybir.dt.uint16
Alu = mybir.AluOpType
Act = mybir.ActivationFunctionType
AxX = mybir.AxisListType.X

NKK = KT // 2         # 28 double-row contraction pairs
RES_SCALE = 2.0 ** -15  # residual psum scale: xl*2^10 @ w*2^5 and x*2^-2 @ wl*2^17

last_run = {}


def _build(mode):
    nc = bacc.Bacc("TRN2", target_bir_lowering=False, debug=False,
                   num_devices=NCORES)

    if mode == "bf16x3":
        xhi = nc.dram_tensor("xhi", [NT, P, DIM], bf16, kind="ExternalInput").ap()
        xlo = nc.dram_tensor("xlo", [NT, P, DIM], bf16, kind="ExternalInput").ap()
        whi = nc.dram_tensor("whi", [P, KT * E], bf16, kind="ExternalInput").ap()
        wlo = nc.dram_tensor("wlo", [P, KT * E], bf16, kind="ExternalInput").ap()
    else:
        xdt = f32r if mode == "fp32r" else f32
        xt = nc.dram_tensor("xt", [NT, P, DIM], xdt, kind="ExternalInput").ap()
        wt = nc.dram_tensor("wt", [P, KT * E], xdt, kind="ExternalInput").ap()
    biasb = nc.dram_tensor("biasb", [P, E], f32, kind="ExternalInput").ap()
    w_out = nc.dram_tensor("w_out", [TPC, TOPK], f32, kind="ExternalOutput").ap()
    idx_out = nc.dram_tensor("idx_out", [TPC, TOPK], i32, kind="ExternalOutput").ap()

    with tile.TileContext(nc) as tc:
        with (
            tc.tile_pool(name="const", bufs=1) as const,
            tc.tile_pool(name="xp", bufs=3) as xp,
            tc.tile_pool(name="ps", bufs=6, space="PSUM") as psp,
            tc.tile_pool(name="rt", bufs=3) as rt,
        ):
            # PE warmup: dummy matmuls on zeroed scratch with no DMA deps —
            # trips the HAM clock gate to 2.4 GHz while the first tiles
            # stream in
            warm_sb = const.tile([P, E], bf16, tag="warm")
            nc.vector.memset(warm_sb[:], 0.0)
            with tc.tile_pool(name="warmps", bufs=1, space="PSUM") as wpsp:
                warm_ps = wpsp.tile([P, E], f32)
                NWARM = 32
                for i in range(NWARM):
                    nc.tensor.matmul(warm_ps[:], warm_sb[:, :P], warm_sb[:],
                                     start=(i == 0), stop=(i == NWARM - 1))

            # weight + tile-0 x loads, chunked and interleaved in K order so
            # the k=0 operands land first and matmuls start ~4us in; each
            # DMA stays on one queue so per-matmul waits stay within ISA
            # limits
            WCH = 8   # K-tiles per weight DMA chunk
            XCH = 8 * P  # x free-dim elements per DMA chunk
            if mode == "bf16x3":
                whi_t = const.tile([P, KT * E], bf16)
                wlo_t = const.tile([P, KT * E], bf16)
                xh0 = xp.tile([P, DIM], bf16, tag="xh")
                xl0 = xp.tile([P, DIM], bf16, tag="xl")
                for c in range(0, KT, WCH):
                    wsl = slice(c * E, (c + WCH) * E)
                    xsl = slice(c * P, (c + WCH) * P)
                    nc.sync.dma_start(whi_t[:, wsl], whi[:, wsl])
                    nc.sync.dma_start(wlo_t[:, wsl], wlo[:, wsl])
                    nc.sync.dma_start(xh0[:, xsl], xhi[0][:, xsl])
                    nc.sync.dma_start(xl0[:, xsl], xlo[0][:, xsl])
            else:
                wt_t = const.tile([P, KT * E], xdt)
                xf0 = xp.tile([P, DIM], xdt, tag="x")
                for c in range(0, KT, WCH):
                    wsl = slice(c * E, (c + WCH) * E)
                    xsl = slice(c * P, (c + WCH) * P)
                    nc.sync.dma_start(wt_t[:, wsl], wt[:, wsl])
                    nc.sync.dma_start(xf0[:, xsl], xt[0][:, xsl])
            bias_t = const.tile([P, E], f32)
            nc.sync.dma_start(bias_t[:], biasb)

            for tt in range(NT):
                # ---- load x tile (partition = contraction dim), chunked ----
                if mode == "bf16x3":
                    if tt == 0:
                        xh, xl = xh0, xl0
                    else:
                        xh = xp.tile([P, DIM], bf16, tag="xh")
                        xl = xp.tile([P, DIM], bf16, tag="xl")
                        for c in range(0, DIM, XCH):
                            sl = slice(c, c + XCH)
                            nc.sync.dma_start(xh[:, sl], xhi[tt][:, sl])
                            nc.sync.dma_start(xl[:, sl], xlo[tt][:, sl])
                else:
                    if tt == 0:
                        xf = xf0
                    else:
                        xf = xp.tile([P, DIM], xdt, tag="x")
                        for c in range(0, DIM, XCH):
                            sl = slice(c, c + XCH)
                            nc.sync.dma_start(xf[:, sl], xt[tt][:, sl])

                # ---- logits: accumulate over 56 K-tiles into PSUM ----
                ps = psp.tile([P, E], f32)
                if mode == "bf16x3":
                    nmm = 3 * KT
                    i = 0
                    for k in range(KT):
                        for xs, ws in ((xh, whi_t), (xh, wlo_t), (xl, whi_t)):
                            nc.tensor.matmul(
                                ps[:],
                                xs[:, k * P:(k + 1) * P],
                                ws[:, k * E:(k + 1) * E],
                                start=(i == 0), stop=(i == nmm - 1),
                            )
                            i += 1
                else:
                    for k in range(KT):
                        nc.tensor.matmul(ps[:], xf[:, k * P:(k + 1) * P],
                                         wt_t[:, k * E:(k + 1) * E],
                                         start=(k == 0), stop=(k == KT - 1))

                # ---- sigmoid (PSUM -> SBUF) ----
                orig = rt.tile([P, E], f32, tag="orig")
                nc.scalar.activation(orig[:], ps[:], Act.Sigmoid)

                # ---- selection scores s = orig + bias ----
                s = rt.tile([P, E], f32, tag="s")
                nc.vector.tensor_tensor(s[:], orig[:], bias_t[:], Alu.add)
                s3 = s[:].rearrange("p (g j) -> p g j", g=G)

                # ---- per-group top-2 sum ----
                m1 = rt.tile([P, G], f32, tag="m1")
                nc.vector.tensor_reduce(m1[:], s3, AxX, Alu.max)
                srep = rt.tile([P, E], f32, tag="srep")
                nc.vector.match_replace(srep[:], m1[:], s[:], NEG)
                m2 = rt.tile([P, G], f32, tag="m2")
                nc.vector.tensor_reduce(
                    m2[:], srep[:].rearrange("p (g j) -> p g j", g=G), AxX, Alu.max)
                gs = rt.tile([P, G], f32, tag="gs")
                nc.vector.tensor_tensor(gs[:], m1[:], m2[:], Alu.add)

                # ---- top-4 groups: threshold = 4th largest group score ----
                gtop = rt.tile([P, 8], f32, tag="gtop")
                nc.vector.max(gtop[:], gs[:])
                km = rt.tile([P, G], f32, tag="km")  # 0 for kept, NEG for dropped
                nc.vector.tensor_scalar(
                    km[:], gs[:], gtop[:, TOPKG - 1:TOPKG], NEG,
                    op0=Alu.is_lt, op1=Alu.mult)

                # ---- mask dropped groups: smask = s + km[group] ----
                smask = rt.tile([P, E], f32, tag="smask")
                for g in range(G):
                    nc.vector.tensor_scalar(
                        smask[:, g * EPG:(g + 1) * EPG],
                        s[:, g * EPG:(g + 1) * EPG],
                        km[:, g:g + 1], None, op0=Alu.add)

                # ---- expert top-8 values + indices ----
                v8 = rt.tile([P, TOPK], f32, tag="v8")
                nc.vector.max(v8[:], smask[:])
                i8u = rt.tile([P, TOPK], u16, tag="i8u")
                nc.vector.max_index(i8u[:], v8[:], smask[:])

                # ---- gather original scores at the 8 winners ----
                w8r = rt.tile([P, TOPK], f32, tag="w8r")
                for k in range(TOPK):
                    tmp = rt.tile([P, E], f32, tag="tmp")
                    nc.vector.scalar_tensor_tensor(
                        tmp[:], smask[:], v8[:, k:k + 1], orig[:],
                        op0=Alu.is_equal, op1=Alu.mult,
                        accum_out=w8r[:, k:k + 1])

                # ---- normalize * SCALE ----
                ssum = rt.tile([P, 1], f32, tag="ssum")
                nc.vector.tensor_reduce(ssum[:], w8r[:], AxX, Alu.add)
                rec = rt.tile([P, 1], f32, tag="rec")
                nc.vector.reciprocal(rec[:], ssum[:])
                w8 = rt.tile([P, TOPK], f32, tag="w8")
                nc.vector.tensor_scalar(
                    w8[:], w8r[:], rec[:, 0:1], SCALE,
                    op0=Alu.mult, op1=Alu.mult)
                i8 = rt.tile([P, TOPK], i32, tag="i8")
                nc.vector.tensor_copy(i8[:], i8u[:])

                nc.sync.dma_start(w_out[tt * P:(tt + 1) * P, :], w8[:])
                nc.sync.dma_start(idx_out[tt * P:(tt + 1) * P, :], i8[:])

    nc.compile()
    return nc


def _routing(nc, tc, rt, orig, bias_t, w_out, idx_out, tt):
    """Grouped top-k routing for one 128-token tile, from sigmoid scores."""
    s = rt.tile([P, E], f32, tag="s")
    nc.vector.tensor_tensor(s[:], orig[:], bias_t[:], Alu.add)
    s3 = s[:].rearrange("p (g j) -> p g j", g=G)

    m1 = rt.tile([P, G], f32, tag="m1")
    nc.vector.tensor_reduce(m1[:], s3, AxX, Alu.max)
    srep = rt.tile([P, E], f32, tag="srep")
    nc.vector.match_replace(srep[:], m1[:], s[:], NEG)
    m2 = rt.tile([P, G], f32, tag="m2")
    nc.vector.tensor_reduce(
        m2[:], srep[:].rearrange("p (g j) -> p g j", g=G), AxX, Alu.max)
    gs = rt.tile([P, G], f32, tag="gs")
    nc.vector.tensor_tensor(gs[:], m1[:], m2[:], Alu.add)

    gtop = rt.tile([P, 8], f32, tag="gtop")
    nc.vector.max(gtop[:], gs[:])
    km = rt.tile([P, G], f32, tag="km")
    nc.vector.tensor_scalar(
        km[:], gs[:], gtop[:, TOPKG - 1:TOPKG], NEG,
        op0=Alu.is_lt, op1=Alu.mult)

    smask = rt.tile([P, E], f32, tag="smask")
    for g in range(G):
        nc.vector.tensor_scalar(
            smask[:, g * EPG:(g + 1) * EPG],
            s[:, g * EPG:(g + 1) * EPG],
            km[:, g:g + 1], None, op0=Alu.add)

    v8 = rt.tile([P, TOPK], f32, tag="v8")
    nc.vector.max(v8[:], smask[:])
    i8u = rt.tile([P, TOPK], u16, tag="i8u")
    nc.vector.max_index(i8u[:], v8[:], smask[:])

    w8r = rt.tile([P, TOPK], f32, tag="w8r")
    for k in range(TOPK):
        tmp = rt.tile([P, E], f32, tag="tmp")
        nc.vector.scalar_tensor_tensor(
            tmp[:], smask[:], v8[:, k:k + 1], orig[:],
            op0=Alu.is_equal, op1=Alu.mult,
            accum_out=w8r[:, k:k + 1])

    ssum = rt.tile([P, 1], f32, tag="ssum")
    nc.vector.tensor_reduce(ssum[:], w8r[:], AxX, Alu.add)
    rec = rt.tile([P, 1], f32, tag="rec")
    nc.vector.reciprocal(rec[:], ssum[:])
    w8 = rt.tile([P, TOPK], f32, tag="w8")
    nc.vector.tensor_scalar(
        w8[:], w8r[:], rec[:, 0:1], SCALE,
        op0=Alu.mult, op1=Alu.mult)
    i8 = rt.tile([P, TOPK], i32, tag="i8")
    nc.vector.tensor_copy(i8[:], i8u[:])

    nc.sync.dma_start(w_out[tt * P:(tt + 1) * P, :], w8[:])
    nc.sync.dma_start(idx_out[tt * P:(tt + 1) * P, :], i8[:])


def _build_hybrid():
    """fp16 main matmul + fp8e4 DoubleRow residual passes.

    logits = xh16 @ wh16 + 2^-15 * (xl8 @ w8 + x8 @ wl8)
    where xh16=fp16(x), xl8=fp8((x-xh16)*2^10), x8=fp8(x*2^-2),
          wh16=fp16(w), w8=fp8(w*2^5),  wl8=fp8((w-wh16)*2^17).
    """
    nc = bacc.Bacc("TRN2", target_bir_lowering=False, debug=False,
                   num_devices=NCORES)

    xh = nc.dram_tensor("xh", [NT, P, DIM], f16, kind="ExternalInput").ap()
    xl8 = nc.dram_tensor("xl8", [NT, P, DIM], f8e4, kind="ExternalInput").ap()
    x8d = nc.dram_tensor("x8d", [NT, P, DIM], f8e4, kind="ExternalInput").ap()
    wh = nc.dram_tensor("wh", [P, KT * E], f16, kind="ExternalInput").ap()
    w8d = nc.dram_tensor("w8d", [P, KT * E], f8e4, kind="ExternalInput").ap()
    wl8d = nc.dram_tensor("wl8d", [P, KT * E], f8e4, kind="ExternalInput").ap()
    biasb = nc.dram_tensor("biasb", [P, E], f32, kind="ExternalInput").ap()
    w_out = nc.dram_tensor("w_out", [TPC, TOPK], f32, kind="ExternalOutput").ap()
    idx_out = nc.dram_tensor("idx_out", [TPC, TOPK], i32, kind="ExternalOutput").ap()

    with tile.TileContext(nc) as tc:
        with (
            tc.tile_pool(name="const", bufs=1) as const,
            tc.tile_pool(name="xp", bufs=3) as xp,
            tc.tile_pool(name="ps", bufs=3, space="PSUM") as psp,
            tc.tile_pool(name="rt", bufs=3) as rt,
        ):
            # PE warmup (clock-gate ramp) while first tiles stream in
            warm_sb = const.tile([P, E], bf16, tag="warm")
            nc.vector.memset(warm_sb[:], 0.0)
            with tc.tile_pool(name="warmps", bufs=1, space="PSUM") as wpsp:
                warm_ps = wpsp.tile([P, E], f32)
                NWARM = 32
                for i in range(NWARM):
                    nc.tensor.matmul(warm_ps[:], warm_sb[:, :P], warm_sb[:],
                                     start=(i == 0), stop=(i == NWARM - 1))

            # weights + x tile 0, chunked + interleaved so k=0 operands land
            # first and matmuls start early
            wh_t = const.tile([P, KT * E], f16)
            w8_t = const.tile([P, KT * E], f8e4)
            wl8_t = const.tile([P, KT * E], f8e4)
            xh0 = xp.tile([P, DIM], f16, tag="xh")
            xl80 = xp.tile([P, DIM], f8e4, tag="xl8")
            x80 = xp.tile([P, DIM], f8e4, tag="x8")
            WCH = 8 * E    # weight elems per chunk (8 k-tiles)
            XCH = 8 * P    # x elems per chunk
            for c in range(KT // 8):
                wsl = slice(c * WCH, (c + 1) * WCH)
                xsl = slice(c * XCH, (c + 1) * XCH)
                nc.sync.dma_start(wh_t[:, wsl], wh[:, wsl])
                nc.sync.dma_start(w8_t[:, wsl], w8d[:, wsl])
                nc.sync.dma_start(wl8_t[:, wsl], wl8d[:, wsl])
                nc.sync.dma_start(xh0[:, xsl], xh[0][:, xsl])
                nc.sync.dma_start(xl80[:, xsl], xl8[0][:, xsl])
                nc.sync.dma_start(x80[:, xsl], x8d[0][:, xsl])
            bias_t = const.tile([P, E], f32)
            nc.sync.dma_start(bias_t[:], biasb)

            for tt in range(NT):
                if tt == 0:
                    xh_t, xl8_t, x8_t = xh0, xl80, x80
                else:
                    xh_t = xp.tile([P, DIM], f16, tag="xh")
                    xl8_t = xp.tile([P, DIM], f8e4, tag="xl8")
                    x8_t = xp.tile([P, DIM], f8e4, tag="x8")
                    for c in range(0, DIM, XCH):
                        sl = slice(c, c + XCH)
                        nc.sync.dma_start(xh_t[:, sl], xh[tt][:, sl])
                        nc.sync.dma_start(xl8_t[:, sl], xl8[tt][:, sl])
                        nc.sync.dma_start(x8_t[:, sl], x8d[tt][:, sl])

                # main pass: 56 fp16 matmuls
                ps_m = psp.tile([P, E], f32, tag="psm")
                for k in range(KT):
                    nc.tensor.matmul(ps_m[:], xh_t[:, k * P:(k + 1) * P],
                                     wh_t[:, k * E:(k + 1) * E],
                                     start=(k == 0), stop=(k == KT - 1))

                # residual pass: 2x28 fp8 DoubleRow matmuls (contraction 256)
                ps_r = psp.tile([P, E], f32, tag="psr")
                i = 0
                nres = 2 * NKK
                for xs, ws in ((xl8_t, w8_t), (x8_t, wl8_t)):
                    for kk in range(NKK):
                        nc.tensor.matmul(
                            ps_r[:],
                            xs[:, kk * 256:(kk + 1) * 256].rearrange(
                                "p (j m) -> p j m", j=2),
                            ws[:, kk * 512:(kk + 1) * 512].rearrange(
                                "p (j n) -> p j n", j=2),
                            start=(i == 0), stop=(i == nres - 1),
                            perf_mode=mybir.MatmulPerfMode.DoubleRow,
                        )
                        i += 1

                # combine + sigmoid (only one PSUM operand allowed per op)
                res_sb = rt.tile([P, E], f32, tag="res")
                nc.scalar.activation(res_sb[:], ps_r[:], Act.Copy,
                                     scale=RES_SCALE)
                logit = rt.tile([P, E], f32, tag="logit")
                nc.vector.tensor_tensor(logit[:], res_sb[:], ps_m[:], Alu.add)
                orig = rt.tile([P, E], f32, tag="orig")
                nc.scalar.activation(orig[:], logit[:], Act.Sigmoid)

                _routing(nc, tc, rt, orig, bias_t, w_out, idx_out, tt)

    nc.compile()
    return nc


def _shuffle_x_dr(xc):
    """[TPC, DIM] -> [NT, P, DIM] DoubleRow layout:
    out[tt, p, kk*256 + j*128 + t] = xc[tt*128 + t, kk*256 + j*128 + p]."""
    return np.ascontiguousarray(
        xc.reshape(NT, P, NKK, 2, P).transpose(0, 4, 2, 3, 1).reshape(NT, P, DIM))


def _shuffle_w_dr(w):
    """[E, DIM] -> [P, NKK*2*E]:
    out[p, kk*512 + j*256 + e] = w[e, kk*256 + j*128 + p]."""
    return np.ascontiguousarray(
        w.T.reshape(NKK, 2, P, E).transpose(2, 0, 1, 3).reshape(P, KT * E))


def _shuffle_x(xc):
    """[TPC, DIM] -> [NT, P, DIM] with out[tt, p, k*128+j] = xc[tt*128+j, k*128+p]."""
    return np.ascontiguousarray(
        xc.reshape(NT, P, KT, P).transpose(0, 3, 2, 1).reshape(NT, P, DIM))


def _shuffle_w(w):
    """[E, DIM] -> [P, KT*E] with out[p, k*E+e] = w[e, k*128+p]."""
    return np.ascontiguousarray(
        w.T.reshape(KT, P, E).transpose(1, 0, 2).reshape(P, KT * E))


_nc_cache = {}


def kernel(x, weight, bias):
    import ml_dtypes

    x = np.asarray(x, dtype=np.float32)
    weight = np.asarray(weight, dtype=np.float32)
    bias = np.asarray(bias, dtype=np.float32)

    mode = MODE
    if mode not in _nc_cache:
        _nc_cache[mode] = (_build_hybrid() if mode == "hybrid"
                           else _build(mode))
    nc = _nc_cache[mode]

    biasb = np.ascontiguousarray(np.broadcast_to(bias, (P, E)))
    in_maps = []
    if mode == "hybrid":
        f8 = ml_dtypes.float8_e4m3
        w_h = weight.astype(np.float16)
        w_l = weight - w_h.astype(np.float32)
        wh = _shuffle_w(w_h.astype(np.float32)).astype(np.float16)
        w8 = _shuffle_w_dr(weight * 2.0 ** 5).astype(f8)
        wl8 = _shuffle_w_dr(w_l * 2.0 ** 17).astype(f8)
        for c in range(NCORES):
            xc = x[c * TPC:(c + 1) * TPC]
            x_h = xc.astype(np.float16)
            x_l = xc - x_h.astype(np.float32)
            in_maps.append({
                "xh": _shuffle_x(x_h.astype(np.float32)).astype(np.float16),
                "xl8": _shuffle_x_dr(x_l * 2.0 ** 10).astype(f8),
                "x8d": _shuffle_x_dr(xc * 2.0 ** -2).astype(f8),
                "wh": wh, "w8d": w8, "wl8d": wl8, "biasb": biasb,
            })
    elif mode == "bf16x3":
        w_hi = weight.astype(ml_dtypes.bfloat16)
        w_lo = (weight - w_hi.astype(np.float32)).astype(ml_dtypes.bfloat16)
        whi = _shuffle_w(w_hi.astype(np.float32)).astype(ml_dtypes.bfloat16)
        wlo = _shuffle_w(w_lo.astype(np.float32)).astype(ml_dtypes.bfloat16)
        for c in range(NCORES):
            xc = x[c * TPC:(c + 1) * TPC]
            x_hi = xc.astype(ml_dtypes.bfloat16)
            x_lo = (xc - x_hi.astype(np.float32)).astype(ml_dtypes.bfloat16)
            in_maps.append({
                "xhi": _shuffle_x(x_hi.astype(np.float32)).astype(ml_dtypes.bfloat16),
                "xlo": _shuffle_x(x_lo.astype(np.float32)).astype(ml_dtypes.bfloat16),
                "whi": whi, "wlo": wlo, "biasb": biasb,
            })
    else:
        wt = _shuffle_w(weight)
        for c in range(NCORES):
            xc = x[c * TPC:(c + 1) * TPC]
            in_maps.append({"xt": _shuffle_x(xc), "wt": wt, "biasb": biasb})

    trace = bool(int(os.environ.get("GATE_KERNEL_TRACE", "0")))
    res = run_bass_kernel_spmd(nc, in_maps, core_ids=list(range(NCORES)),
                               trace=trace)
    last_run["exec_time_ns"] = res.exec_time_ns
    last_run["mean_exec_time_ns"] = res.mean_exec_time_ns
    last_run["trace"] = res.instructions_and_trace

    w = np.concatenate([res.results[c]["w_out"] for c in range(NCORES)], axis=0)
    idx = np.concatenate([res.results[c]["idx_out"] for c in range(NCORES)], axis=0)
    return w.astype(np.float32), idx.astype(np.int32)



# revision 9
# speedup vs baseline: 1.4207x; 1.4207x over previous
"""MoE gate (sigmoid scores + grouped top-k routing) on 8 Trainium2 cores.

Reference computation (per token):
    scores = sigmoid(x @ W.T)                  # [T, 256]
    s = scores + bias                          # selection scores
    group_score[g] = sum(top2(s[g*32:(g+1)*32]))
    keep top-4 groups, mask the rest to -inf
    idx = top8(masked s)                       # [T, 8] int32, descending
    w = scores[idx]; w = w / w.sum() * 2.5     # [T, 8] f32

Sharding: tokens split 8 ways (2048/core); W/bias replicated. Host
pre-shuffles x and W into the transposed tiled layout the TensorE needs
(contraction dim on partitions) so the device does no transposes.

Device per 128-token tile: 56 accumulating matmuls (K=128 each) into
PSUM -> sigmoid on ScalarE -> routing (grouped top-2 via segmented
reduce_max + match_replace, group top-4 via max8, expert top-8 via
max8/find_index8) on VectorE. Final score gather is 8 fused
compare-multiply-accumulate ops.
"""

import os

import numpy as np

import concourse.bass as bass
import concourse.mybir as mybir
import concourse.tile as tile
from concourse import bacc
from concourse.bass_utils import run_bass_kernel_spmd

T = 16384
DIM = 7168
E = 256
G = 8
EPG = E // G          # 32 experts per group
TOPKG = 4
TOPK = 8
SCALE = 2.5
NCORES = 8
TPC = T // NCORES     # 2048 tokens per core
P = 128
NT = TPC // P         # 16 token tiles per core
KT = DIM // P         # 56 contraction tiles
NEG = -1.0e30

# matmul precision: 'fp32r' (1 cyc/row, tf32-ish), 'bf16x3' (3 cyc/row,
# near-fp32), 'fp32' (4 cyc/row, exact)
MODE = os.environ.get("GATE_KERNEL_MODE", "bf16x3")

f32 = mybir.dt.float32
f32r = mybir.dt.float32r
bf16 = mybir.dt.bfloat16
f16 = mybir.dt.float16
f8e4 = mybir.dt.float8e4
i32 = mybir.dt.int32
u16 = mybir.dt.uint16
Alu = mybir.AluOpType
Act = mybir.ActivationFunctionType
AxX = mybir.AxisListType.X

NKK = KT // 2         # 28 double-row contraction pairs
RES_SCALE = 2.0 ** -15  # residual psum scale: (xl*2^10)@(w*2^5) and x@(wl*2^15)

last_run = {}


def _build(mode):
    nc = bacc.Bacc("TRN2", target_bir_lowering=False, debug=False,
                   num_devices=NCORES)

    if mode == "bf16x3":
        xhi = nc.dram_tensor("xhi", [NT, P, DIM], bf16, kind="ExternalInput").ap()
        xlo = nc.dram_tensor("xlo", [NT, P, DIM], bf16, kind="ExternalInput").ap()
        whi = nc.dram_tensor("whi", [P, KT * E], bf16, kind="ExternalInput").ap()
        wlo = nc.dram_tensor("wlo", [P, KT * E], bf16, kind="ExternalInput").ap()
    else:
        xdt = f32r if mode == "fp32r" else f32
        xt = nc.dram_tensor("xt", [NT, P, DIM], xdt, kind="ExternalInput").ap()
        wt = nc.dram_tensor("wt", [P, KT * E], xdt, kind="ExternalInput").ap()
    biasb = nc.dram_tensor("biasb", [P, E], f32, kind="ExternalInput").ap()
    w_out = nc.dram_tensor("w_out", [TPC, TOPK], f32, kind="ExternalOutput").ap()
    idx_out = nc.dram_tensor("idx_out", [TPC, TOPK], i32, kind="ExternalOutput").ap()

    with tile.TileContext(nc) as tc:
        with (
            tc.tile_pool(name="const", bufs=1) as const,
            tc.tile_pool(name="xp", bufs=3) as xp,
            tc.tile_pool(name="ps", bufs=6, space="PSUM") as psp,
            tc.tile_pool(name="rt", bufs=3) as rt,
        ):
            # PE warmup: dummy matmuls on zeroed scratch with no DMA deps —
            # trips the HAM clock gate to 2.4 GHz while the first tiles
            # stream in
            warm_sb = const.tile([P, E], bf16, tag="warm")
            nc.vector.memset(warm_sb[:], 0.0)
            with tc.tile_pool(name="warmps", bufs=1, space="PSUM") as wpsp:
                warm_ps = wpsp.tile([P, E], f32)
                NWARM = 32
                for i in range(NWARM):
                    nc.tensor.matmul(warm_ps[:], warm_sb[:, :P], warm_sb[:],
                                     start=(i == 0), stop=(i == NWARM - 1))

            # weight + tile-0 x loads, chunked and interleaved in K order so
            # the k=0 operands land first and matmuls start ~4us in; each
            # DMA stays on one queue so per-matmul waits stay within ISA
            # limits
            WCH = 8   # K-tiles per weight DMA chunk
            XCH = 8 * P  # x free-dim elements per DMA chunk
            if mode == "bf16x3":
                whi_t = const.tile([P, KT * E], bf16)
                wlo_t = const.tile([P, KT * E], bf16)
                xh0 = xp.tile([P, DIM], bf16, tag="xh")
                xl0 = xp.tile([P, DIM], bf16, tag="xl")
                for c in range(0, KT, WCH):
                    wsl = slice(c * E, (c + WCH) * E)
                    xsl = slice(c * P, (c + WCH) * P)
                    nc.sync.dma_start(whi_t[:, wsl], whi[:, wsl])
                    nc.sync.dma_start(wlo_t[:, wsl], wlo[:, wsl])
                    nc.sync.dma_start(xh0[:, xsl], xhi[0][:, xsl])
                    nc.sync.dma_start(xl0[:, xsl], xlo[0][:, xsl])
            else:
                wt_t = const.tile([P, KT * E], xdt)
                xf0 = xp.tile([P, DIM], xdt, tag="x")
                for c in range(0, KT, WCH):
                    wsl = slice(c * E, (c + WCH) * E)
                    xsl = slice(c * P, (c + WCH) * P)
                    nc.sync.dma_start(wt_t[:, wsl], wt[:, wsl])
                    nc.sync.dma_start(xf0[:, xsl], xt[0][:, xsl])
            bias_t = const.tile([P, E], f32)
            nc.sync.dma_start(bias_t[:], biasb)

            for tt in range(NT):
                # ---- load x tile (partition = contraction dim), chunked ----
                if mode == "bf16x3":
                    if tt == 0:
                        xh, xl = xh0, xl0
                    else:
                        xh = xp.tile([P, DIM], bf16, tag="xh")
                        xl = xp.tile([P, DIM], bf16, tag="xl")
                        for c in range(0, DIM, XCH):
                            sl = slice(c, c + XCH)
                            nc.sync.dma_start(xh[:, sl], xhi[tt][:, sl])
                            nc.sync.dma_start(xl[:, sl], xlo[tt][:, sl])
                else:
                    if tt == 0:
                        xf = xf0
                    else:
                        xf = xp.tile([P, DIM], xdt, tag="x")
                        for c in range(0, DIM, XCH):
                            sl = slice(c, c + XCH)
                            nc.sync.dma_start(xf[:, sl], xt[tt][:, sl])

                # ---- logits: accumulate over 56 K-tiles into PSUM ----
                ps = psp.tile([P, E], f32)
                if mode == "bf16x3":
                    nmm = 3 * KT
                    i = 0
                    for k in range(KT):
                        for xs, ws in ((xh, whi_t), (xh, wlo_t), (xl, whi_t)):
                            nc.tensor.matmul(
                                ps[:],
                                xs[:, k * P:(k + 1) * P],
                                ws[:, k * E:(k + 1) * E],
                                start=(i == 0), stop=(i == nmm - 1),
                            )
                            i += 1
                else:
                    for k in range(KT):
                        nc.tensor.matmul(ps[:], xf[:, k * P:(k + 1) * P],
                                         wt_t[:, k * E:(k + 1) * E],
                                         start=(k == 0), stop=(k == KT - 1))

                # ---- sigmoid (PSUM -> SBUF) ----
                orig = rt.tile([P, E], f32, tag="orig")
                nc.scalar.activation(orig[:], ps[:], Act.Sigmoid)

                # ---- selection scores s = orig + bias ----
                s = rt.tile([P, E], f32, tag="s")
                nc.vector.tensor_tensor(s[:], orig[:], bias_t[:], Alu.add)
                s3 = s[:].rearrange("p (g j) -> p g j", g=G)

                # ---- per-group top-2 sum ----
                m1 = rt.tile([P, G], f32, tag="m1")
                nc.vector.tensor_reduce(m1[:], s3, AxX, Alu.max)
                srep = rt.tile([P, E], f32, tag="srep")
                nc.vector.match_replace(srep[:], m1[:], s[:], NEG)
                m2 = rt.tile([P, G], f32, tag="m2")
                nc.vector.tensor_reduce(
                    m2[:], srep[:].rearrange("p (g j) -> p g j", g=G), AxX, Alu.max)
                gs = rt.tile([P, G], f32, tag="gs")
                nc.vector.tensor_tensor(gs[:], m1[:], m2[:], Alu.add)

                # ---- top-4 groups: threshold = 4th largest group score ----
                gtop = rt.tile([P, 8], f32, tag="gtop")
                nc.vector.max(gtop[:], gs[:])
                km = rt.tile([P, G], f32, tag="km")  # 0 for kept, NEG for dropped
                nc.vector.tensor_scalar(
                    km[:], gs[:], gtop[:, TOPKG - 1:TOPKG], NEG,
                    op0=Alu.is_lt, op1=Alu.mult)

                # ---- mask dropped groups: smask = s + km[group] ----
                smask = rt.tile([P, E], f32, tag="smask")
                for g in range(G):
                    nc.vector.tensor_scalar(
                        smask[:, g * EPG:(g + 1) * EPG],
                        s[:, g * EPG:(g + 1) * EPG],
                        km[:, g:g + 1], None, op0=Alu.add)

                # ---- expert top-8 values + indices ----
                v8 = rt.tile([P, TOPK], f32, tag="v8")
                nc.vector.max(v8[:], smask[:])
                i8u = rt.tile([P, TOPK], u16, tag="i8u")
                nc.vector.max_index(i8u[:], v8[:], smask[:])

                # ---- gather original scores at the 8 winners ----
                w8r = rt.tile([P, TOPK], f32, tag="w8r")
                for k in range(TOPK):
                    tmp = rt.tile([P, E], f32, tag="tmp")
                    nc.vector.scalar_tensor_tensor(
                        tmp[:], smask[:], v8[:, k:k + 1], orig[:],
                        op0=Alu.is_equal, op1=Alu.mult,
                        accum_out=w8r[:, k:k + 1])

                # ---- normalize * SCALE ----
                ssum = rt.tile([P, 1], f32, tag="ssum")
                nc.vector.tensor_reduce(ssum[:], w8r[:], AxX, Alu.add)
                rec = rt.tile([P, 1], f32, tag="rec")
                nc.vector.reciprocal(rec[:], ssum[:])
                w8 = rt.tile([P, TOPK], f32, tag="w8")
                nc.vector.tensor_scalar(
                    w8[:], w8r[:], rec[:, 0:1], SCALE,
                    op0=Alu.mult, op1=Alu.mult)
                i8 = rt.tile([P, TOPK], i32, tag="i8")
                nc.vector.tensor_copy(i8[:], i8u[:])

                nc.sync.dma_start(w_out[tt * P:(tt + 1) * P, :], w8[:])
                nc.sync.dma_start(idx_out[tt * P:(tt + 1) * P, :], i8[:])

    nc.compile()
    return nc


def _routing(nc, tc, rt, orig, bias_t, w_out, idx_out, tt):
    """Grouped top-k routing for one 128-token tile, from sigmoid scores."""
    s = rt.tile([P, E], f32, tag="s")
    nc.vector.tensor_tensor(s[:], orig[:], bias_t[:], Alu.add)
    s3 = s[:].rearrange("p (g j) -> p g j", g=G)

    m1 = rt.tile([P, G], f32, tag="m1")
    nc.vector.tensor_reduce(m1[:], s3, AxX, Alu.max)
    srep = rt.tile([P, E], f32, tag="srep")
    nc.vector.match_replace(srep[:], m1[:], s[:], NEG)
    m2 = rt.tile([P, G], f32, tag="m2")
    nc.vector.tensor_reduce(
        m2[:], srep[:].rearrange("p (g j) -> p g j", g=G), AxX, Alu.max)
    gs = rt.tile([P, G], f32, tag="gs")
    nc.vector.tensor_tensor(gs[:], m1[:], m2[:], Alu.add)

    gtop = rt.tile([P, 8], f32, tag="gtop")
    nc.vector.max(gtop[:], gs[:])
    km = rt.tile([P, G], f32, tag="km")
    nc.vector.tensor_scalar(
        km[:], gs[:], gtop[:, TOPKG - 1:TOPKG], NEG,
        op0=Alu.is_lt, op1=Alu.mult)

    smask = rt.tile([P, E], f32, tag="smask")
    for g in range(G):
        nc.vector.tensor_scalar(
            smask[:, g * EPG:(g + 1) * EPG],
            s[:, g * EPG:(g + 1) * EPG],
            km[:, g:g + 1], None, op0=Alu.add)

    v8 = rt.tile([P, TOPK], f32, tag="v8")
    nc.vector.max(v8[:], smask[:])
    i8u = rt.tile([P, TOPK], u16, tag="i8u")
    nc.vector.max_index(i8u[:], v8[:], smask[:])

    w8r = rt.tile([P, TOPK], f32, tag="w8r")
    for k in range(TOPK):
        tmp = rt.tile([P, E], f32, tag="tmp")
        nc.vector.scalar_tensor_tensor(
            tmp[:], smask[:], v8[:, k:k + 1], orig[:],
            op0=Alu.is_equal, op1=Alu.mult,
            accum_out=w8r[:, k:k + 1])

    ssum = rt.tile([P, 1], f32, tag="ssum")
    nc.vector.tensor_reduce(ssum[:], w8r[:], AxX, Alu.add)
    rec = rt.tile([P, 1], f32, tag="rec")
    nc.vector.reciprocal(rec[:], ssum[:])
    w8 = rt.tile([P, TOPK], f32, tag="w8")
    nc.vector.tensor_scalar(
        w8[:], w8r[:], rec[:, 0:1], SCALE,
        op0=Alu.mult, op1=Alu.mult)
    i8 = rt.tile([P, TOPK], i32, tag="i8")
    nc.vector.tensor_copy(i8[:], i8u[:])

    nc.sync.dma_start(w_out[tt * P:(tt + 1) * P, :], w8[:])
    nc.sync.dma_start(idx_out[tt * P:(tt + 1) * P, :], i8[:])


def _build_hybrid():
    """fp16 main matmul + fp8e4 DoubleRow residual passes.

    logits = xh16 @ wh16 + 2^-15 * (xl8 @ w8 + x8 @ wl8)
    where xh16=fp16(x), xl8=fp8((x-xh16)*2^10), x8=fp8(x*2^-2),
          wh16=fp16(w), w8=fp8(w*2^5),  wl8=fp8((w-wh16)*2^17).
    """
    nc = bacc.Bacc("TRN2", target_bir_lowering=False, debug=False,
                   num_devices=NCORES)

    xh = nc.dram_tensor("xh", [NT, P, DIM], f16, kind="ExternalInput").ap()
    xl8 = nc.dram_tensor("xl8", [NT, P, DIM], f8e4, kind="ExternalInput").ap()
    wh = nc.dram_tensor("wh", [P, KT * E], f16, kind="ExternalInput").ap()
    w8d = nc.dram_tensor("w8d", [P, KT * E], f8e4, kind="ExternalInput").ap()
    wl8d = nc.dram_tensor("wl8d", [P, KT * E], f8e4, kind="ExternalInput").ap()
    biasb = nc.dram_tensor("biasb", [P, E], f32, kind="ExternalInput").ap()
    w_out = nc.dram_tensor("w_out", [TPC, TOPK], f32, kind="ExternalOutput").ap()
    idx_out = nc.dram_tensor("idx_out", [TPC, TOPK], i32, kind="ExternalOutput").ap()

    with tile.TileContext(nc) as tc:
        with (
            tc.tile_pool(name="const", bufs=1) as const,
            tc.tile_pool(name="xp", bufs=4) as xp,
            tc.tile_pool(name="x8p", bufs=3) as x8p,
            tc.tile_pool(name="ps", bufs=3, space="PSUM") as psp,
            tc.tile_pool(name="rt", bufs=3) as rt,
        ):
            # PE warmup (clock-gate ramp) while first tiles stream in
            warm_sb = const.tile([P, E], bf16, tag="warm")
            nc.vector.memset(warm_sb[:], 0.0)
            with tc.tile_pool(name="warmps", bufs=1, space="PSUM") as wpsp:
                warm_ps = wpsp.tile([P, E], f32)
                NWARM = 32
                for i in range(NWARM):
                    nc.tensor.matmul(warm_ps[:], warm_sb[:, :P], warm_sb[:],
                                     start=(i == 0), stop=(i == NWARM - 1))

            # weights + x tile 0, chunked + interleaved so k=0 operands land
            # first and matmuls start early
            wh_t = const.tile([P, KT * E], f16)
            w8_t = const.tile([P, KT * E], f8e4)
            wl8_t = const.tile([P, KT * E], f8e4)
            xh0 = xp.tile([P, DIM], f16, tag="xh")
            xl80 = xp.tile([P, DIM], f8e4, tag="xl8")
            WCH = 8 * E    # weight elems per chunk (8 k-tiles)
            XCH = 8 * P    # x elems per chunk
            for c in range(KT // 8):
                wsl = slice(c * WCH, (c + 1) * WCH)
                xsl = slice(c * XCH, (c + 1) * XCH)
                nc.sync.dma_start(wh_t[:, wsl], wh[:, wsl])
                nc.sync.dma_start(w8_t[:, wsl], w8d[:, wsl])
                nc.sync.dma_start(wl8_t[:, wsl], wl8d[:, wsl])
                nc.sync.dma_start(xh0[:, xsl], xh[0][:, xsl])
                nc.sync.dma_start(xl80[:, xsl], xl8[0][:, xsl])
            bias_t = const.tile([P, E], f32)
            nc.sync.dma_start(bias_t[:], biasb)

            for tt in range(NT):
                if tt == 0:
                    xh_t, xl8_t = xh0, xl80
                else:
                    xh_t = xp.tile([P, DIM], f16, tag="xh")
                    xl8_t = xp.tile([P, DIM], f8e4, tag="xl8")
                    h = DIM // 2
                    nc.sync.dma_start(xh_t[:, :h], xh[tt][:, :h])
                    nc.sync.dma_start(xh_t[:, h:], xh[tt][:, h:])
                    nc.sync.dma_start(xl8_t[:], xl8[tt][:])
                # derive x8 = fp8(xh) on the Act engine (saves 14.7MB of DMA)
                x8_t = x8p.tile([P, DIM], f8e4, tag="x8")
                nc.scalar.activation(x8_t[:], xh_t[:], Act.Copy)

                # main pass: 56 fp16 matmuls
                ps_m = psp.tile([P, E], f32, tag="psm")
                for k in range(KT):
                    nc.tensor.matmul(ps_m[:], xh_t[:, k * P:(k + 1) * P],
                                     wh_t[:, k * E:(k + 1) * E],
                                     start=(k == 0), stop=(k == KT - 1))

                # residual pass: 2x28 fp8 DoubleRow matmuls (contraction 256)
                ps_r = psp.tile([P, E], f32, tag="psr")
                i = 0
                nres = 2 * NKK
                for xs, ws in ((xl8_t, w8_t), (x8_t, wl8_t)):
                    for kk in range(NKK):
                        nc.tensor.matmul(
                            ps_r[:],
                            xs[:, kk * 256:(kk + 1) * 256].rearrange(
                                "p (j m) -> p j m", j=2),
                            ws[:, kk * 512:(kk + 1) * 512].rearrange(
                                "p (j n) -> p j n", j=2),
                            start=(i == 0), stop=(i == nres - 1),
                            perf_mode=mybir.MatmulPerfMode.DoubleRow,
                        )
                        i += 1

                # combine + sigmoid (only one PSUM operand allowed per op)
                res_sb = rt.tile([P, E], f32, tag="res")
                nc.scalar.activation(res_sb[:], ps_r[:], Act.Copy,
                                     scale=RES_SCALE)
                logit = rt.tile([P, E], f32, tag="logit")
                nc.vector.tensor_tensor(logit[:], res_sb[:], ps_m[:], Alu.add)
                orig = rt.tile([P, E], f32, tag="orig")
                nc.scalar.activation(orig[:], logit[:], Act.Sigmoid)

                _routing(nc, tc, rt, orig, bias_t, w_out, idx_out, tt)

    nc.compile()
    return nc


def _shuffle_x_dr(xc):
    """[TPC, DIM] -> [NT, P, DIM] DoubleRow layout:
    out[tt, p, kk*256 + j*128 + t] = xc[tt*128 + t, kk*256 + j*128 + p]."""
    return np.ascontiguousarray(
        xc.reshape(NT, P, NKK, 2, P).transpose(0, 4, 2, 3, 1).reshape(NT, P, DIM))


def _shuffle_w_dr(w):
    """[E, DIM] -> [P, NKK*2*E]:
    out[p, kk*512 + j*256 + e] = w[e, kk*256 + j*128 + p]."""
    return np.ascontiguousarray(
        w.T.reshape(NKK, 2, P, E).transpose(2, 0, 1, 3).reshape(P, KT * E))


def _shuffle_x(xc):
    """[TPC, DIM] -> [NT, P, DIM] with out[tt, p, k*128+j] = xc[tt*128+j, k*128+p]."""
    return np.ascontiguousarray(
        xc.reshape(NT, P, KT, P).transpose(0, 3, 2, 1).reshape(NT, P, DIM))


def _shuffle_w(w):
    """[E, DIM] -> [P, KT*E] with out[p, k*E+e] = w[e, k*128+p]."""
    return np.ascontiguousarray(
        w.T.reshape(KT, P, E).transpose(1, 0, 2).reshape(P, KT * E))


_nc_cache = {}


def kernel(x, weight, bias):
    import ml_dtypes

    x = np.asarray(x, dtype=np.float32)
    weight = np.asarray(weight, dtype=np.float32)
    bias = np.asarray(bias, dtype=np.float32)

    mode = MODE
    if mode not in _nc_cache:
        _nc_cache[mode] = (_build_hybrid() if mode == "hybrid"
                           else _build(mode))
    nc = _nc_cache[mode]

    biasb = np.ascontiguousarray(np.broadcast_to(bias, (P, E)))
    in_maps = []
    if mode == "hybrid":
        f8 = ml_dtypes.float8_e4m3
        w_h = weight.astype(np.float16)
        w_l = weight - w_h.astype(np.float32)
        wh = _shuffle_w(w_h.astype(np.float32)).astype(np.float16)
        w8 = _shuffle_w_dr(weight * 2.0 ** 5).astype(f8)
        wl8 = _shuffle_w_dr(w_l * 2.0 ** 15).astype(f8)
        for c in range(NCORES):
            xc = x[c * TPC:(c + 1) * TPC]
            x_h = xc.astype(np.float16)
            x_l = xc - x_h.astype(np.float32)
            in_maps.append({
                "xh": _shuffle_x(x_h.astype(np.float32)).astype(np.float16),
                "xl8": _shuffle_x_dr(x_l * 2.0 ** 10).astype(f8),
                "wh": wh, "w8d": w8, "wl8d": wl8, "biasb": biasb,
            })
    elif mode == "bf16x3":
        w_hi = weight.astype(ml_dtypes.bfloat16)
        w_lo = (weight - w_hi.astype(np.float32)).astype(ml_dtypes.bfloat16)
        whi = _shuffle_w(w_hi.astype(np.float32)).astype(ml_dtypes.bfloat16)
        wlo = _shuffle_w(w_lo.astype(np.float32)).astype(ml_dtypes.bfloat16)
        for c in range(NCORES):
            xc = x[c * TPC:(c + 1) * TPC]
            x_hi = xc.astype(ml_dtypes.bfloat16)
            x_lo = (xc - x_hi.astype(np.float32)).astype(ml_dtypes.bfloat16)
            in_maps.append({
                "xhi": _shuffle_x(x_hi.astype(np.float32)).astype(ml_dtypes.bfloat16),
                "xlo": _shuffle_x(x_lo.astype(np.float32)).astype(ml_dtypes.bfloat16),
                "whi": whi, "wlo": wlo, "biasb": biasb,
            })
    else:
        wt = _shuffle_w(weight)
        for c in range(NCORES):
            xc = x[c * TPC:(c + 1) * TPC]
            in_maps.append({"xt": _shuffle_x(xc), "wt": wt, "biasb": biasb})

    trace = bool(int(os.environ.get("GATE_KERNEL_TRACE", "0")))
    res = run_bass_kernel_spmd(nc, in_maps, core_ids=list(range(NCORES)),
                               trace=trace)
    last_run["exec_time_ns"] = res.exec_time_ns
    last_run["mean_exec_time_ns"] = res.mean_exec_time_ns
    last_run["trace"] = res.instructions_and_trace

    w = np.concatenate([res.results[c]["w_out"] for c in range(NCORES)], axis=0)
    idx = np.concatenate([res.results[c]["idx_out"] for c in range(NCORES)], axis=0)
    return w.astype(np.float32), idx.astype(np.int32)

